# revision 4
# baseline (speedup 1.0000x reference)
"""Trainium2 Bass kernel for a 5x5 conv2d (NCHW, pad=2, stride=1).

Problem: X [32,32,128,128] f32, K [64,32,5,5] f32 -> out [32,64,128,128].
Sharding: data-parallel over 8 NeuronCores, 4 images per core.

Per-core mapping:
  The 4 images of the shard occupy the 4 PE row-groups (SBUF partitions
  32g..32g+31 hold image g's 32 input channels). Each conv tap (dy,dx)
  of each image is one K=32 x M=64 matmul whose rhs is an access-pattern
  offset into a zero-padded band of the image held in SBUF. Inputs are
  converted to bf16 on the host: bf16 enables PE column tiling, so
  4 row-groups x 2 col-groups = 8 concurrent 32x64 matmuls cover all
  16 32x32 PE sub-arrays (col group j computes output-row block j of an
  8-row window). The 25 taps accumulate in PSUM (fp32).

  DMA traffic in the steady state is minimized: each band arrives as ONE
  DMA (all 4 images, contiguous partition dim), the four band buffers are
  persistent and pre-zeroed once so pad rows/columns never need reloading,
  and each psum round leaves through one shared staging tile and 2 DMAs.
"""

import numpy as np
import ml_dtypes

import concourse.bass as bass
import concourse.tile as tile
from concourse import bacc, mybir
from concourse.bass_utils import run_bass_kernel_spmd

N_CORES = 8
IMGS = 4          # images per core = PE row groups
C = 32            # input channels
O = 64            # output channels
H = W = 128
KH = KW = 5
PAD = 2
WP = W + 2 * PAD  # 132 padded row length
BANDS = 4
BAND_OUT = H // BANDS         # 32 output rows per band
BAND_IN = BAND_OUT + 2 * PAD  # 36 stored padded rows per band
TAPS = KH * KW    # 25
RT = 4            # output rows per col-group block (RT*W = 512 = max N)
BLK = 2 * RT      # output rows per psum round (2 col groups)

F32 = mybir.dt.float32
MM_DT = mybir.dt.bfloat16
NP_DT = ml_dtypes.bfloat16


def _build_nc(reps=1):
    nc = bacc.Bacc("TRN2", target_bir_lowering=False, debug=False)
    # all 4 images stacked on the partition axis: partition 32g+c holds
    # image g, channel c
    X = nc.dram_tensor("X", [IMGS * C, H, W], MM_DT, kind="ExternalInput").ap()
    K = nc.dram_tensor("K", [C, TAPS, O], MM_DT, kind="ExternalInput").ap()
    ZF = nc.dram_tensor(
        "ZF", [128, BAND_IN, WP], MM_DT, kind="ExternalInput"
    ).ap()
    out = nc.dram_tensor("out", [IMGS, O, H, W], F32, kind="ExternalOutput").ap()

    with tile.TileContext(nc) as tc:
        with (
            tc.tile_pool(name="wpool", bufs=1) as wpool,
            tc.tile_pool(name="xpool", bufs=1) as xpool,
            tc.tile_pool(name="opool", bufs=4) as opool,
            tc.tile_pool(name="ppool", bufs=8, space="PSUM") as ppool,
        ):
            # Weights: partition 32g+c holds K[o, c, tap] for image-group g
            # (same copy in each of the 4 partition groups so every PE
            # row-group can load its stationary operand locally).
            wt = wpool.tile([128, TAPS, O], MM_DT)
            for g in range(IMGS):
                nc.sync.dma_start(wt[32 * g : 32 * g + 32, :, :], K)

            # Persistent band buffers, one per band, zeroed once: the X
            # band DMA only ever writes the interior, so pad rows/columns
            # stay zero across all iterations.
            xbs = [
                xpool.tile([128, BAND_IN, WP], MM_DT, name=f"xb{b}")
                for b in range(BANDS)
            ]
            for b in range(BANDS):
                nc.sync.dma_start(xbs[b][:, :, :], ZF)

            tap_order = [(2, 2)] + [
                (dy, dx)
                for dy in range(KH)
                for dx in range(KW)
                if (dy, dx) != (2, 2)
            ]

            def body():
              for b in range(BANDS):
                y0 = b * BAND_OUT  # first output row; padded rows y0..y0+35
                xb = xbs[b]
                # stored position p holds real input row y0 + p - PAD
                p_lo = PAD if b == 0 else 0
                p_hi = BAND_IN - 1 - PAD if b == BANDS - 1 else BAND_IN - 1
                r_lo = y0 + p_lo - PAD
                r_hi = y0 + p_hi - PAD
                # one DMA for the whole band: all 4 images at once
                nc.sync.dma_start(
                    xb[:, p_lo : p_hi + 1, PAD : PAD + W],
                    X[:, r_lo : r_hi + 1, :],
                )

                # 4 psum rounds per band; round t accumulates output rows
                # y0+8t..+7 for each of the 4 images: col group 0 computes
                # the first 4 rows (psum partitions 0..63), col group 1 the
                # next 4 (psum partitions 64..127). 4 images x 2 col groups
                # = 8 concurrent 32x64-tile matmuls = full PE array.
                for t in range(BAND_OUT // BLK):
                    pss = [
                        ppool.tile(
                            [128, RT, W], F32, name=f"ps_b{b}_t{t}_g{g}", tag="ps"
                        )
                        for g in range(IMGS)
                    ]
                    ybase = BLK * t
                    gy = y0 + ybase
                    for ti, (dy, dx) in enumerate(tap_order):
                        first = ti == 0
                        last = ti == TAPS - 1
                        tap = dy * KW + dx
                        for g in range(IMGS):
                            lhsT = wt[32 * g : 32 * g + 32, tap, :]
                            for j in range(2):
                                rhs = xb[
                                    32 * g : 32 * g + 32,
                                    ybase + RT * j + dy : ybase + RT * j + dy + RT,
                                    dx : dx + W,
                                ]
                                nc.tensor.matmul(
                                    pss[g][64 * j : 64 * j + 64, :, :],
                                    lhsT,
                                    rhs,
                                    start=first,
                                    stop=last,
                                    tile_position=(32 * g, 64 * j),
                                )
                    # evacuate: one shared staging tile, then 2 DMAs (one
                    # per col group) instead of 8
                    ob = opool.tile([128, IMGS, RT, W], F32)
                    for g in range(IMGS):
                        nc.vector.tensor_copy(ob[:, g, :, :], pss[g][:, :, :])
                    for j in range(2):
                        nc.sync.dma_start(
                            out[:, :, gy + RT * j : gy + RT * (j + 1), :]
                            .rearrange("g o r w -> o g r w"),
                            ob[64 * j : 64 * j + 64, :, :, :],
                        )

            if reps > 1:
                with tc.For_i(0, reps, 1):
                    body()
            else:
                body()
    nc.compile()
    return nc


_CACHE = {}


def _get_nc(reps=1):
    if reps not in _CACHE:
        _CACHE[reps] = _build_nc(reps)
    return _CACHE[reps]


def make_in_maps(X, K):
    """Shard full inputs into per-core input maps (host-side prep)."""
    X = np.asarray(X, dtype=np.float32)
    K = np.asarray(K, dtype=np.float32)
    per = X.shape[0] // N_CORES
    Kb = np.ascontiguousarray(
        K.reshape(O, C, TAPS).transpose(1, 2, 0).astype(NP_DT)
    )
    ZF = np.zeros((128, BAND_IN, WP), dtype=NP_DT)
    return [
        {
            "X": np.ascontiguousarray(
                X[per * i : per * (i + 1)].reshape(IMGS * C, H, W)
            ).astype(NP_DT),
            "K": Kb,
            "ZF": ZF,
        }
        for i in range(N_CORES)
    ]


def kernel(X, K):
    nc = _get_nc()
    in_maps = make_in_maps(X, K)
    res = run_bass_kernel_spmd(nc, in_maps, list(range(N_CORES))).results
    return np.concatenate([res[i]["out"] for i in range(N_CORES)], axis=0)
